# revision 17
# baseline (speedup 1.0000x reference)
"""Expert-parallel MoE kernel for Trainium2 (8 NeuronCores).

Strategy (matches the module's intent):
  - Host computes the (tiny) gating: logits -> softmax -> top-2 -> renormalized
    combine weights. This is the router / all-to-all dispatch plumbing.
  - Expert e's weights (W1[e], b1[e], W2[e], b2[e]) live on core e.
  - Core e receives only its routed tokens (transposed, bf16) plus their
    combine weights, and computes  w * (gelu(x @ W1e + b1e) @ W2e + b2e)
    entirely on device (both matmuls in bf16 with fp32 PSUM accumulation).
  - Host scatter-adds the per-expert partial outputs back (the combine).

Layout: activations are kept feature-major on device (features on SBUF
partitions, tokens on the free dim) so both weight matrices are used in
their native layout as the stationary matmul operand and no transposes
are needed anywhere on device.
"""

import sys

sys.path.insert(0, "/opt/trn_rl_repo")

import numpy as np
import ml_dtypes

H = 768
E = 8
DFF = 3072
P = 128
HO = H // P      # 6 h-tiles
FO = DFF // P    # 24 f-tiles
N_CORES = 8
N_WARMUP_MM = 64  # dummy matmuls to open the HAM clock gate during DMA ramp
FBLK_COLS = 512   # w1 arrives in f-blocks of this many columns
NFBLK = DFF // FBLK_COLS
JPB = FBLK_COLS // P  # f-tiles (j) per block

_COMPILED = {}       # (C, TS) -> compiled Bacc program
LAST_RESULTS = None  # BassKernelResults of the most recent run (for test.py)
TRACE = False        # set True (e.g. by test.py) to profile the run


def _token_slices(C):
    n_t = -(-C // 512)
    base = -(-C // (n_t * 32)) * 32
    sizes = []
    left = C
    for _ in range(n_t):
        s = min(base, left)
        if s > 0:
            sizes.append(s)
        left -= s
    return tuple(sizes)


def _build(C, TS, act="gelu"):
    import concourse.bass as bass
    import concourse.mybir as mybir
    import concourse.tile as tile
    from concourse import bacc

    f32 = mybir.dt.float32
    bf16 = mybir.dt.bfloat16
    GELU = (
        mybir.ActivationFunctionType.Gelu
        if act == "gelu"
        else mybir.ActivationFunctionType.Identity
    )
    IDENT = mybir.ActivationFunctionType.Identity

    nc = bacc.Bacc("TRN2", target_bir_lowering=False, debug=False)

    # Host passes everything pre-tiled so each DMA source is one contiguous
    # per-partition segment (max-size descriptors, minimal push cost).
    xT_d = nc.dram_tensor("xT", [P, HO, C], bf16, kind="ExternalInput").ap()
    w1_d = nc.dram_tensor(
        "w1", [NFBLK, P, HO, FBLK_COLS], bf16, kind="ExternalInput"
    ).ap()
    w2_d = nc.dram_tensor("w2", [P, FO, H], bf16, kind="ExternalInput").ap()
    b1_d = nc.dram_tensor("b1", [P, FO], f32, kind="ExternalInput").ap()
    b2_d = nc.dram_tensor("b2", [P, HO], f32, kind="ExternalInput").ap()
    wr_d = nc.dram_tensor("wr", [1, C], f32, kind="ExternalInput").ap()
    out_d = nc.dram_tensor("outT", [H, C], f32, kind="ExternalOutput").ap()

    with tile.TileContext(nc) as tc:
        with (
            tc.tile_pool(name="const", bufs=1) as const,
            tc.tile_pool(name="hmidp", bufs=1) as hmidp,
            tc.tile_pool(name="psum", bufs=6, space="PSUM") as psum,
            tc.tile_pool(name="wupp", bufs=1, space="PSUM") as wupp,
            tc.tile_pool(name="outp", bufs=4) as outp,
        ):
            # ---- PE warm-up: dummy matmuls so the HAM clock-gate opens while
            # the weight DMAs are still in flight --------------------------------
            scr = const.tile([P, P], bf16, name="scr", tag="scr")
            nc.vector.memset(scr, 0.0)
            psd = wupp.tile([P, P], f32, name="psd", tag="psd")
            for _ in range(N_WARMUP_MM):
                nc.tensor.matmul(psd, lhsT=scr, rhs=scr, start=True, stop=True)

            # ---- small operands -------------------------------------------------
            b1_sb = const.tile([P, FO], f32, name="b1_sb", tag="b1_sb")
            nc.scalar.dma_start(out=b1_sb, in_=b1_d)
            b2_sb = const.tile([P, HO], f32, name="b2_sb", tag="b2_sb")
            nc.scalar.dma_start(out=b2_sb, in_=b2_d)

            # combine weights, broadcast across all 128 partitions
            wb_sb = const.tile([P, C], f32, name="wb_sb", tag="wb_sb")
            wr_bcast = bass.AP(
                tensor=wr_d.tensor, offset=wr_d.offset, ap=[[0, P], [1, C]]
            )
            nc.gpsimd.dma_start(out=wb_sb, in_=wr_bcast)


            # ---- input / weight loads (issue order == DMA priority) ------------
            # xT in one push: [128, HO, C]
            xT_sb = const.tile([P, HO, C], bf16, name="xT", tag="xT")
            nc.sync.dma_start(out=xT_sb, in_=xT_d)

            # w1 chunked along f but spanning all ho: first chunk unblocks the
            # first JPB psum groups completely. [128, HO, FBLK_COLS] each.
            w1_sb = []
            for fb in range(NFBLK):
                t = const.tile(
                    [P, HO, FBLK_COLS], bf16, name=f"w1_{fb}", tag=f"w1_{fb}"
                )
                nc.sync.dma_start(out=t, in_=w1_d[fb])
                w1_sb.append(t)

            # w2 in one push on the scalar HWDGE ring: [128, FO, H]
            w2_sb = const.tile([P, FO, H], bf16, name="w2", tag="w2")
            nc.scalar.dma_start(out=w2_sb, in_=w2_d)

            hmid_sb = [
                hmidp.tile([P, C], bf16, name=f"hmid{fo}", tag=f"hmid{fo}")
                for fo in range(FO)
            ]

            # ---- MLP layer 1: hmidT[f, t] = gelu(sum_h W1[h,f] xT[h,t] + b1[f]) --
            starts = np.cumsum([0] + list(TS))
            for ti, tn in enumerate(TS):
                t0 = int(starts[ti])
                for j in range(FO):
                    fb, jj = divmod(j, JPB)
                    ps = psum.tile([P, 512], f32, name="ps1", tag="ps")
                    for ho in range(HO):
                        nc.tensor.matmul(
                            ps[:, :tn],
                            lhsT=w1_sb[fb][:, ho, jj * P : (jj + 1) * P],
                            rhs=xT_sb[:, ho, t0 : t0 + tn],
                            start=(ho == 0),
                            stop=(ho == HO - 1),
                        )
                    nc.scalar.activation(
                        hmid_sb[j][:, t0 : t0 + tn],
                        ps[:, :tn],
                        GELU,
                        bias=b1_sb[:, j : j + 1],
                    )

            # ---- MLP layer 2 + combine scale ------------------------------------
            for ti, tn in enumerate(TS):
                t0 = int(starts[ti])
                for i in range(HO):
                    ps = psum.tile([P, 512], f32, name="ps2", tag="ps")
                    for fo in range(FO):
                        nc.tensor.matmul(
                            ps[:, :tn],
                            lhsT=w2_sb[:, fo, i * P : (i + 1) * P],
                            rhs=hmid_sb[fo][:, t0 : t0 + tn],
                            start=(fo == 0),
                            stop=(fo == FO - 1),
                        )
                    ot = outp.tile([P, 512], f32, name="ot", tag="ot")
                    nc.scalar.activation(
                        ot[:, :tn], ps[:, :tn], IDENT, bias=b2_sb[:, i : i + 1]
                    )
                    nc.vector.tensor_mul(
                        ot[:, :tn], ot[:, :tn], wb_sb[:, t0 : t0 + tn]
                    )
                    nc.sync.dma_start(
                        out=out_d[i * P : (i + 1) * P, t0 : t0 + tn], in_=ot[:, :tn]
                    )

    nc.compile()
    return nc


def kernel(x, Wg, bg, W1, b1, W2, b2, top_k):
    global LAST_RESULTS
    from concourse import bass_utils

    x = np.asarray(x, dtype=np.float32)
    Wg = np.asarray(Wg, dtype=np.float32)
    bg = np.asarray(bg, dtype=np.float32)
    W1 = np.asarray(W1, dtype=np.float32)
    b1 = np.asarray(b1, dtype=np.float32)
    W2 = np.asarray(W2, dtype=np.float32)
    b2 = np.asarray(b2, dtype=np.float32)
    k = int(np.asarray(top_k))
    assert k == 2, f"kernel specialized for top_k=2, got {k}"

    b, s, h = x.shape
    T = b * s
    xf = x.reshape(T, h)

    # ---- host router (the all-to-all dispatch) ------------------------------
    logits = xf @ Wg + bg
    m = logits.max(axis=-1, keepdims=True)
    p = np.exp(logits - m)
    p /= p.sum(axis=-1, keepdims=True)
    i1 = np.argmax(p, axis=-1)
    p_masked = p.copy()
    p_masked[np.arange(T), i1] = -np.inf
    i2 = np.argmax(p_masked, axis=-1)
    denom = p[np.arange(T), i1] + p[np.arange(T), i2]

    tok_idx, tok_w = [], []
    for e in range(E):
        sel = np.where((i1 == e) | (i2 == e))[0]
        tok_idx.append(sel.astype(np.int64))
        tok_w.append((p[sel, e] / denom[sel]).astype(np.float32))
    max_cnt = max(len(t) for t in tok_idx)
    C = max(-(-max_cnt // 32) * 32, 128)
    TS = _token_slices(C)

    key = (C, TS)
    if key not in _COMPILED:
        _COMPILED[key] = _build(C, TS)
    nc = _COMPILED[key]

    # ---- per-core inputs ----------------------------------------------------
    bf = ml_dtypes.bfloat16
    in_maps = []
    for e in range(E):
        cnt = len(tok_idx[e])
        # xT pre-tiled [P, HO, C]: xT[p, o, c] = x[token c, feature o*P+p]
        xg = np.zeros((P, HO, C), dtype=bf)
        xg[:, :, :cnt] = (
            np.ascontiguousarray(xf[tok_idx[e]].T).astype(bf)
            .reshape(HO, P, cnt)
            .transpose(1, 0, 2)
        )
        wr = np.zeros((1, C), dtype=np.float32)
        wr[0, :cnt] = tok_w[e]
        # w1 [NFBLK, P, HO, FBLK]: w1[fb, p, o, f] = W1[o*P+p, fb*FBLK+f]
        w1t = np.ascontiguousarray(
            W1[e].astype(bf)
            .reshape(HO, P, NFBLK, FBLK_COLS)
            .transpose(2, 1, 0, 3)
        )
        # w2 [P, FO, H]: w2[p, o, h] = W2[o*P+p, h]
        w2t = np.ascontiguousarray(
            W2[e].astype(bf).reshape(FO, P, H).transpose(1, 0, 2)
        )
        in_maps.append(
            {
                "xT": xg,
                "w1": w1t,
                "w2": w2t,
                "b1": np.ascontiguousarray(b1[e].reshape(FO, P).T),
                "b2": np.ascontiguousarray(b2[e].reshape(HO, P).T),
                "wr": wr,
            }
        )

    res = bass_utils.run_bass_kernel_spmd(
        nc, in_maps, core_ids=list(range(N_CORES)), trace=TRACE
    )
    LAST_RESULTS = res

    # ---- combine (scatter-add of the weighted expert outputs) ---------------
    out = np.zeros((T, H), dtype=np.float32)
    for e in range(E):
        cnt = len(tok_idx[e])
        if cnt:
            out[tok_idx[e]] += res.results[e]["outT"][:, :cnt].T
    return out.reshape(b, s, h)
